# revision 4
# baseline (speedup 1.0000x reference)
"""GCN link-prediction kernel for 8 Trainium2 NeuronCores.

Strategy (target-sharded edges, replicated-by-AllGather node tables):
  - Nodes split into 8 contiguous shards. Each core computes its shard of
    g1 = dinv * (x @ W1) on PE, AllGather -> full table gtab1 in every
    core's HBM.
  - Train edges assigned to the core owning the TARGET node, grouped per
    128-target tile, padded to a fixed chunk count. Per 128-edge chunk:
    indirect-DMA gather of source rows, a DVE is_equal indicator matrix
    [edges x targets], and one PE matmul accumulating into PSUM.
    Self-loops (weight 2) are a per-tile extra chunk whose indicator is
    doubled.
  - Layer epilogue fuses dinv scaling, bias, relu, and the next layer's
    dense matmul (transposed via PE) so h1/h2 never round-trip to DRAM.
  - Edge head: z = h2 @ Wl1 table (64 f32), two gathers per 128-edge
    chunk, elementwise ops + free-dim reduction + sigmoid.

All float math runs on device in fp32; host only reorders/pads integer
edge indices and computes dinv (index-derived degree scaling).
"""
import sys
import os
import numpy as np

sys.path.insert(0, '/opt/trn_rl_repo')

N_CORES = 8
N = 50000
F_IN, H1, H2, H3 = 256, 256, 128, 64
SELF_LOOP_W = 2.0

NS = N // N_CORES            # 6250 nodes per shard
NT = (NS + 127) // 128       # 49 target tiles per core
NSP = NT * 128               # 6272 padded shard rows
HEAD_E = 400000
HE_CORE = HEAD_E // N_CORES  # 50000 head edges per core
NHC = (HE_CORE + 127) // 128  # 391 head chunks

_CACHE = {}


def _build_and_compile(C):
    """Build the SPMD Bass program with C data-chunks per target tile."""
    import concourse.bass as bass
    import concourse.mybir as mybir
    import concourse.tile as tile
    from concourse import bacc

    dt = mybir.dt
    CH = C + 1            # +1 self chunk (chunk 0)
    CHT = NT * CH         # total chunks per core

    nc = bacc.Bacc("TRN2", target_bir_lowering=False, debug=False,
                   num_devices=N_CORES)

    # ---- I/O ----
    xT = nc.dram_tensor("xT", [F_IN, NSP], dt.float32, kind="ExternalInput")
    dinv_pm = nc.dram_tensor("dinv_pm", [128, NT], dt.float32, kind="ExternalInput")
    W1 = nc.dram_tensor("W1", [F_IN, H1], dt.float32, kind="ExternalInput")
    W2 = nc.dram_tensor("W2", [H1, H2], dt.float32, kind="ExternalInput")
    Wl1 = nc.dram_tensor("Wl1", [H2, H3], dt.float32, kind="ExternalInput")
    b1t = nc.dram_tensor("b1t", [128, H1], dt.float32, kind="ExternalInput")
    b2t = nc.dram_tensor("b2t", [128, H2], dt.float32, kind="ExternalInput")
    bl1t = nc.dram_tensor("bl1t", [128, H3], dt.float32, kind="ExternalInput")
    wl2bc = nc.dram_tensor("wl2bc", [128, H3], dt.float32, kind="ExternalInput")
    bl2c = nc.dram_tensor("bl2c", [128, 1], dt.float32, kind="ExternalInput")
    esrc = nc.dram_tensor("esrc", [128, CHT], dt.int32, kind="ExternalInput")
    colloc = nc.dram_tensor("colloc", [128, CHT], dt.float32, kind="ExternalInput")
    hsrc0 = nc.dram_tensor("hsrc0", [128, NHC], dt.int32, kind="ExternalInput")
    hsrc1 = nc.dram_tensor("hsrc1", [128, NHC], dt.int32, kind="ExternalInput")
    out_head = nc.dram_tensor("out_head", [128, NHC], dt.float32,
                              kind="ExternalOutput")

    from concourse.masks import make_identity

    with tile.TileContext(nc) as tc:
        with tc.tile_pool(name="const", bufs=1) as cpool, \
             tc.tile_pool(name="dram", bufs=1, space="DRAM") as dpool, \
             tc.tile_pool(name="gat", bufs=8) as gat_pool, \
             tc.tile_pool(name="ind", bufs=8) as ind_pool, \
             tc.tile_pool(name="work", bufs=4) as work, \
             tc.tile_pool(name="psA", bufs=2, space="PSUM") as psA, \
             tc.tile_pool(name="psT", bufs=2, space="PSUM") as psT:

            # ---- constants / index preload ----
            ident = cpool.tile([128, 128], dt.float32)
            make_identity(nc, ident[:])
            iota_i = cpool.tile([128, 128], dt.int32)
            nc.gpsimd.iota(iota_i[:], pattern=[[1, 128]], base=0,
                           channel_multiplier=0)
            iota_f = cpool.tile([128, 128], dt.float32)
            nc.vector.tensor_copy(out=iota_f[:], in_=iota_i[:])

            W1s = cpool.tile([128, 2 * H1], dt.float32)
            nc.sync.dma_start(out=W1s[:, :H1], in_=W1[0:128, :])
            nc.sync.dma_start(out=W1s[:, H1:], in_=W1[128:256, :])
            W2s = cpool.tile([128, 2 * H2], dt.float32)
            nc.sync.dma_start(out=W2s[:, :H2], in_=W2[0:128, :])
            nc.sync.dma_start(out=W2s[:, H2:], in_=W2[128:256, :])
            Wl1s = cpool.tile([128, H3], dt.float32)
            nc.sync.dma_start(out=Wl1s[:], in_=Wl1[:])
            b1s = cpool.tile([128, H1], dt.float32)
            nc.sync.dma_start(out=b1s[:], in_=b1t[:])
            b2s = cpool.tile([128, H2], dt.float32)
            nc.sync.dma_start(out=b2s[:], in_=b2t[:])
            bl1s = cpool.tile([128, H3], dt.float32)
            nc.sync.dma_start(out=bl1s[:], in_=bl1t[:])
            wl2s = cpool.tile([128, H3], dt.float32)
            nc.sync.dma_start(out=wl2s[:], in_=wl2bc[:])
            bl2s = cpool.tile([128, 1], dt.float32)
            nc.sync.dma_start(out=bl2s[:], in_=bl2c[:])
            dinv_s = cpool.tile([128, NT], dt.float32)
            nc.sync.dma_start(out=dinv_s[:], in_=dinv_pm[:])
            esrc_s = cpool.tile([128, CHT], dt.int32)
            nc.sync.dma_start(out=esrc_s[:], in_=esrc[:])
            colloc_s = cpool.tile([128, CHT], dt.float32)
            nc.sync.dma_start(out=colloc_s[:], in_=colloc[:])
            h0_s = cpool.tile([128, NHC], dt.int32)
            nc.sync.dma_start(out=h0_s[:], in_=hsrc0[:])
            h1_s = cpool.tile([128, NHC], dt.int32)
            nc.sync.dma_start(out=h1_s[:], in_=hsrc1[:])

            # ---- DRAM internals ----
            g1_loc = dpool.tile([NS, H1], dt.float32)
            gtab1 = dpool.tile([N, H1], dt.float32, addr_space="Shared")
            g2_loc = dpool.tile([NS, H2], dt.float32)
            gtab2 = dpool.tile([N, H2], dt.float32, addr_space="Shared")
            z_loc = dpool.tile([NS, H3], dt.float32)
            ztab = dpool.tile([N, H3], dt.float32, addr_space="Shared")

            rg = [list(range(N_CORES))]

            # ================= Phase A: g1 shard =================
            for t in range(NT):
                rows = min(128, NS - t * 128)
                xa = work.tile([128, 128], dt.float32, tag="xa")
                xb = work.tile([128, 128], dt.float32, tag="xb")
                nc.sync.dma_start(out=xa[:], in_=xT[0:128, t * 128:(t + 1) * 128])
                nc.sync.dma_start(out=xb[:], in_=xT[128:256, t * 128:(t + 1) * 128])
                ps = psA.tile([128, H1], dt.float32, tag="psagg")
                nc.tensor.matmul(out=ps[:], lhsT=xa[:], rhs=W1s[:, :H1],
                                 start=True, stop=False)
                nc.tensor.matmul(out=ps[:], lhsT=xb[:], rhs=W1s[:, H1:],
                                 start=False, stop=True)
                g1v = work.tile([128, H1], dt.float32, tag="g1v")
                nc.vector.tensor_mul(
                    out=g1v[:], in0=ps[:],
                    in1=dinv_s[:, t:t + 1].to_broadcast([128, H1]))
                nc.sync.dma_start(out=g1_loc[t * 128: t * 128 + rows, :],
                                  in_=g1v[:rows, :])
            nc.gpsimd.collective_compute(
                "AllGather", mybir.AluOpType.bypass, replica_groups=rg,
                ins=[g1_loc.opt()], outs=[gtab1.opt()])

            # ============ Layer helpers ============
            def agg_layer(gtab, F):
                """Yields per-tile psum [128, F] aggregated over CH chunks."""
                for t in range(NT):
                    ps = psA.tile([128, F], dt.float32, tag="psagg")
                    for c in range(CH):
                        j = t * CH + c
                        g = gat_pool.tile([128, F], dt.float32, tag="gath")
                        nc.gpsimd.indirect_dma_start(
                            out=g[:], out_offset=None, in_=gtab[:],
                            in_offset=bass.IndirectOffsetOnAxis(
                                ap=esrc_s[:, j:j + 1], axis=0))
                        ind = ind_pool.tile([128, 128], dt.float32, tag="ind")
                        nc.vector.tensor_tensor(
                            out=ind[:],
                            in0=colloc_s[:, j:j + 1].to_broadcast([128, 128]),
                            in1=iota_f[:], op=mybir.AluOpType.is_equal)
                        if c == 0:
                            nc.vector.tensor_scalar_mul(ind[:], ind[:],
                                                        float(SELF_LOOP_W))
                        nc.tensor.matmul(out=ps[:], lhsT=ind[:], rhs=g[:],
                                         start=(c == 0), stop=(c == CH - 1))
                    yield t, ps

            # ============ Layer 1 + fused g2 ============
            for t, ps in agg_layer(gtab1, H1):
                rows = min(128, NS - t * 128)
                dv = dinv_s[:, t:t + 1]
                h1v = work.tile([128, H1], dt.float32, tag="h1v")
                nc.vector.tensor_mul(out=h1v[:], in0=ps[:],
                                     in1=dv.to_broadcast([128, H1]))
                nc.vector.tensor_add(out=h1v[:], in0=h1v[:], in1=b1s[:])
                nc.scalar.activation(out=h1v[:], in_=h1v[:],
                                     func=mybir.ActivationFunctionType.Relu)
                nc.vector.tensor_mul(out=h1v[:], in0=h1v[:],
                                     in1=dv.to_broadcast([128, H1]))
                # transpose h1d -> [feat, rows], then g2 = h1d @ W2
                g2ps = psA.tile([128, H2], dt.float32, tag="pssm")
                tpss = []
                for fb in range(2):
                    tp = psT.tile([128, 128], dt.float32, tag="tp")
                    nc.tensor.transpose(out=tp[:],
                                        in_=h1v[:, fb * 128:(fb + 1) * 128],
                                        identity=ident[:])
                    tps = work.tile([128, 128], dt.float32, tag=f"tps{fb}")
                    nc.vector.tensor_copy(out=tps[:], in_=tp[:])
                    tpss.append(tps)
                for fb in range(2):
                    nc.tensor.matmul(out=g2ps[:], lhsT=tpss[fb][:],
                                     rhs=W2s[:, fb * H2:(fb + 1) * H2],
                                     start=(fb == 0), stop=(fb == 1))
                g2v = work.tile([128, H2], dt.float32, tag="g2v")
                nc.vector.tensor_copy(out=g2v[:], in_=g2ps[:])
                nc.sync.dma_start(out=g2_loc[t * 128: t * 128 + rows, :],
                                  in_=g2v[:rows, :])
            nc.gpsimd.collective_compute(
                "AllGather", mybir.AluOpType.bypass, replica_groups=rg,
                ins=[g2_loc.opt()], outs=[gtab2.opt()])

            # ============ Layer 2 + fused z ============
            for t, ps in agg_layer(gtab2, H2):
                rows = min(128, NS - t * 128)
                dv = dinv_s[:, t:t + 1]
                h2v = work.tile([128, H2], dt.float32, tag="h2v")
                nc.vector.tensor_mul(out=h2v[:], in0=ps[:],
                                     in1=dv.to_broadcast([128, H2]))
                nc.vector.tensor_add(out=h2v[:], in0=h2v[:], in1=b2s[:])
                tp = psT.tile([128, 128], dt.float32, tag="tp")
                nc.tensor.transpose(out=tp[:], in_=h2v[:], identity=ident[:])
                tps = work.tile([128, 128], dt.float32, tag="tps")
                nc.vector.tensor_copy(out=tps[:], in_=tp[:])
                zps = psA.tile([128, H3], dt.float32, tag="pssm")
                nc.tensor.matmul(out=zps[:], lhsT=tps[:], rhs=Wl1s[:],
                                 start=True, stop=True)
                zv = work.tile([128, H3], dt.float32, tag="zv")
                nc.vector.tensor_copy(out=zv[:], in_=zps[:])
                nc.sync.dma_start(out=z_loc[t * 128: t * 128 + rows, :],
                                  in_=zv[:rows, :])
            nc.gpsimd.collective_compute(
                "AllGather", mybir.AluOpType.bypass, replica_groups=rg,
                ins=[z_loc.opt()], outs=[ztab.opt()])

            # ============ Edge head ============
            out_sb = cpool.tile([128, NHC], dt.float32)
            for c in range(NHC):
                r0 = gat_pool.tile([128, H3], dt.float32, tag="hg0")
                nc.gpsimd.indirect_dma_start(
                    out=r0[:], out_offset=None, in_=ztab[:],
                    in_offset=bass.IndirectOffsetOnAxis(
                        ap=h0_s[:, c:c + 1], axis=0))
                r1 = gat_pool.tile([128, H3], dt.float32, tag="hg1")
                nc.gpsimd.indirect_dma_start(
                    out=r1[:], out_offset=None, in_=ztab[:],
                    in_offset=bass.IndirectOffsetOnAxis(
                        ap=h1_s[:, c:c + 1], axis=0))
                e1 = work.tile([128, H3], dt.float32, tag="e1")
                nc.vector.tensor_add(out=e1[:], in0=r0[:], in1=r1[:])
                nc.vector.tensor_add(out=e1[:], in0=e1[:], in1=bl1s[:])
                nc.scalar.activation(out=e1[:], in_=e1[:],
                                     func=mybir.ActivationFunctionType.Relu)
                nc.vector.tensor_mul(out=e1[:], in0=e1[:], in1=wl2s[:])
                sc = work.tile([128, 1], dt.float32, tag="sc")
                nc.vector.reduce_sum(out=sc[:], in_=e1[:],
                                     axis=mybir.AxisListType.X)
                nc.scalar.activation(out=out_sb[:, c:c + 1], in_=sc[:],
                                     func=mybir.ActivationFunctionType.Sigmoid,
                                     bias=bl2s[:])
            nc.sync.dma_start(out=out_head[:], in_=out_sb[:])

    nc.compile()
    return nc


def _prep_inputs(x, train_edge_index, pos_edge_index, neg_edge_index,
                 W1, b1, W2, b2, Wl1, bl1, Wl2, bl2):
    """Host-side sharding / index layout. Returns (in_maps, C)."""
    x = np.asarray(x, np.float32)
    ei = np.asarray(train_edge_index)
    row, col = ei[0].astype(np.int64), ei[1].astype(np.int64)
    deg = np.bincount(col, minlength=N).astype(np.float32) + SELF_LOOP_W
    dinv = (1.0 / np.sqrt(deg)).astype(np.float32)

    W1 = np.asarray(W1, np.float32)
    W2 = np.asarray(W2, np.float32)
    Wl1 = np.asarray(Wl1, np.float32)
    b1 = np.asarray(b1, np.float32)
    b2 = np.asarray(b2, np.float32)
    bl1 = np.asarray(bl1, np.float32)
    Wl2 = np.asarray(Wl2, np.float32).reshape(-1)
    bl2 = np.asarray(bl2, np.float32).reshape(-1)

    # --- per-(core,tile) edge grouping ---
    core_of = col // NS
    tile_of = (col % NS) // 128
    # chunk requirement per (core, tile)
    counts = np.zeros((N_CORES, NT), np.int64)
    np.add.at(counts, (core_of, tile_of), 1)
    C = max(18, int(np.ceil(counts.max() / 128)))
    CH = C + 1
    CHT = NT * CH

    order = np.lexsort((tile_of, core_of))
    row_s, col_s = row[order], col[order]
    core_s, tile_s = core_of[order], tile_of[order]
    # boundaries per (core,tile)
    grp = core_s * NT + tile_s
    starts = np.searchsorted(grp, np.arange(N_CORES * NT))
    ends = np.searchsorted(grp, np.arange(N_CORES * NT), side='right')

    tei = np.concatenate([np.asarray(pos_edge_index),
                          np.asarray(neg_edge_index)], axis=-1)
    t0_all, t1_all = tei[0].astype(np.int64), tei[1].astype(np.int64)

    in_maps = []
    for k in range(N_CORES):
        lo = k * NS
        esrc = np.zeros((128, CHT), np.int32)
        colloc = np.full((128, CHT), -1.0, np.float32)
        for t in range(NT):
            rows_t = min(128, NS - t * 128)
            base = t * CH
            # self chunk (chunk 0)
            esrc[:rows_t, base] = lo + t * 128 + np.arange(rows_t)
            colloc[:rows_t, base] = np.arange(rows_t)
            # data chunks
            s, e = starts[k * NT + t], ends[k * NT + t]
            ne = e - s
            assert ne <= C * 128, "chunk overflow"
            srcs = row_s[s:e]
            locs = (col_s[s:e] - lo - t * 128).astype(np.float32)
            full = np.zeros(C * 128, np.int32)
            fullc = np.full(C * 128, -1.0, np.float32)
            full[:ne] = srcs
            fullc[:ne] = locs
            esrc[:, base + 1: base + CH] = full.reshape(C, 128).T
            colloc[:, base + 1: base + CH] = fullc.reshape(C, 128).T

        # head edges
        h0 = np.zeros(NHC * 128, np.int32)
        h1 = np.zeros(NHC * 128, np.int32)
        h0[:HE_CORE] = t0_all[k * HE_CORE:(k + 1) * HE_CORE]
        h1[:HE_CORE] = t1_all[k * HE_CORE:(k + 1) * HE_CORE]
        hsrc0 = h0.reshape(NHC, 128).T.copy()
        hsrc1 = h1.reshape(NHC, 128).T.copy()

        # node shard data
        xs = np.zeros((NSP, F_IN), np.float32)
        xs[:NS] = x[lo:lo + NS]
        xT = np.ascontiguousarray(xs.T)
        dpm = np.zeros((128, NT), np.float32)
        dsh = np.zeros(NSP, np.float32)
        dsh[:NS] = dinv[lo:lo + NS]
        dpm[:, :] = dsh.reshape(NT, 128).T

        in_maps.append({
            "xT": xT, "dinv_pm": dpm,
            "W1": W1, "W2": W2, "Wl1": Wl1,
            "b1t": np.tile(b1[None, :], (128, 1)),
            "b2t": np.tile(b2[None, :], (128, 1)),
            "bl1t": np.tile(bl1[None, :], (128, 1)),
            "wl2bc": np.tile(Wl2[None, :], (128, 1)),
            "bl2c": np.full((128, 1), bl2[0], np.float32),
            "esrc": esrc, "colloc": colloc,
            "hsrc0": hsrc0, "hsrc1": hsrc1,
        })
    return in_maps, C


def _get_runner(C, in_maps):
    import jax
    from concourse import bass2jax, mybir as mb
    from jax.sharding import Mesh, PartitionSpec
    from jax.experimental.shard_map import shard_map

    key = ("runner", C)
    if key in _CACHE:
        return _CACHE[key]

    nc = _CACHE.get(("nc", C))
    if nc is None:
        nc = _build_and_compile(C)
        _CACHE[("nc", C)] = nc

    bass2jax.install_neuronx_cc_hook()
    partition_name = nc.partition_id_tensor.name if nc.partition_id_tensor else None
    in_names, out_names, out_avals, zero_outs = [], [], [], []
    for a in nc.m.functions[0].allocations:
        if not isinstance(a, mb.MemoryLocationSet):
            continue
        name = a.memorylocations[0].name
        if a.kind == "ExternalInput":
            if name != partition_name:
                in_names.append(name)
        elif a.kind == "ExternalOutput":
            out_names.append(name)
            shape = tuple(a.tensor_shape)
            dtype = mb.dt.np(a.dtype)
            out_avals.append(jax.core.ShapedArray(shape, dtype))
            zero_outs.append(np.zeros(shape, dtype))
    n_params = len(in_names)
    all_in_names = in_names + out_names + ([partition_name] if partition_name else [])

    def _body(*args):
        operands = list(args)
        if partition_name is not None:
            operands.append(bass2jax.partition_id_tensor())
        outs = bass2jax._bass_exec_p.bind(
            *operands, out_avals=tuple(out_avals), in_names=tuple(all_in_names),
            out_names=tuple(out_names), lowering_input_output_aliases=(),
            sim_require_finite=True, sim_require_nnan=True, nc=nc)
        return tuple(outs)

    devices = jax.devices()[:N_CORES]
    mesh = Mesh(np.asarray(devices), ("core",))
    in_specs = (PartitionSpec("core"),) * (n_params + len(out_names))
    out_specs = (PartitionSpec("core"),) * len(out_names)
    sharded = jax.jit(shard_map(_body, mesh=mesh, in_specs=in_specs,
                                out_specs=out_specs, check_rep=False),
                      keep_unused=True)

    def run(maps):
        concat_in = [np.concatenate([np.asarray(maps[c][nm])
                                     for c in range(N_CORES)], axis=0)
                     for nm in in_names]
        concat_zero = [np.concatenate([z] * N_CORES, axis=0) for z in zero_outs]
        outs = sharded(*concat_in, *concat_zero)
        jax.block_until_ready(outs)
        return {nm: np.asarray(outs[i]) for i, nm in enumerate(out_names)}

    _CACHE[key] = run
    return run


def kernel(**inputs) -> np.ndarray:
    in_maps, C = _prep_inputs(**inputs)
    run = _get_runner(C, in_maps)
    outs = run(in_maps)
    oh = outs["out_head"].reshape(N_CORES, 128, NHC)
    res = np.empty(HEAD_E, np.float32)
    for k in range(N_CORES):
        flat = oh[k].T.reshape(-1)  # [NHC*128] in edge order
        res[k * HE_CORE:(k + 1) * HE_CORE] = flat[:HE_CORE]
    return res


# revision 6
# speedup vs baseline: 152.8271x; 152.8271x over previous
"""GCN link-prediction kernel for 8 Trainium2 NeuronCores.

Strategy (target-sharded edges, replicated-by-AllGather node tables):
  - Nodes split into 8 contiguous shards. Each core computes its shard of
    g1 = dinv * (x @ W1) on PE, AllGather -> full table gtab1 in every
    core's HBM.
  - Train edges assigned to the core owning the TARGET node, grouped per
    128-target tile, padded to a fixed chunk count. Per 128-edge chunk:
    indirect-DMA gather of source rows, a DVE is_equal indicator matrix
    [edges x targets], and one PE matmul accumulating into PSUM.
    Self-loops (weight 2) are a per-tile extra chunk whose indicator is
    doubled.
  - Layer epilogue fuses dinv scaling, bias, relu, and the next layer's
    dense matmul (transposed via PE) so h1/h2 never round-trip to DRAM.
  - Edge head: z = h2 @ Wl1 table (64 f32), two gathers per 128-edge
    chunk, elementwise ops + free-dim reduction + sigmoid.

All float math runs on device in fp32; host only reorders/pads integer
edge indices and computes dinv (index-derived degree scaling).
"""
import sys
import os
import numpy as np

sys.path.insert(0, '/opt/trn_rl_repo')

N_CORES = 8
N = 50000
F_IN, H1, H2, H3 = 256, 256, 128, 64
SELF_LOOP_W = 2.0

NS = N // N_CORES            # 6250 nodes per shard
NT = (NS + 127) // 128       # 49 target tiles per core
NSP = NT * 128               # 6272 padded shard rows
HEAD_E = 400000
HE_CORE = HEAD_E // N_CORES  # 50000 head edges per core
NHC = (HE_CORE + 127) // 128  # 391 head chunks

_CACHE = {}


def _build_and_compile(Cts):
    """Build the SPMD Bass program. Cts[t] = data-chunks for target tile t."""
    import concourse.bass as bass
    import concourse.mybir as mybir
    import concourse.tile as tile
    from concourse import bacc

    dt = mybir.dt
    Cts = list(Cts)
    CHT = sum(Cts)        # data chunks per core (self-loop via direct DMA)
    bases = np.cumsum([0] + Cts)[:-1]

    nc = bacc.Bacc("TRN2", target_bir_lowering=False, debug=False,
                   num_devices=N_CORES)

    # ---- I/O ----
    xT = nc.dram_tensor("xT", [F_IN, NSP], dt.float32, kind="ExternalInput")
    dinv_pm = nc.dram_tensor("dinv_pm", [128, NT], dt.float32, kind="ExternalInput")
    W1 = nc.dram_tensor("W1", [F_IN, H1], dt.float32, kind="ExternalInput")
    W2 = nc.dram_tensor("W2", [H1, H2], dt.float32, kind="ExternalInput")
    Wl1 = nc.dram_tensor("Wl1", [H2, H3], dt.float32, kind="ExternalInput")
    b1t = nc.dram_tensor("b1t", [128, H1], dt.float32, kind="ExternalInput")
    b2t = nc.dram_tensor("b2t", [128, H2], dt.float32, kind="ExternalInput")
    bl1t = nc.dram_tensor("bl1t", [128, H3], dt.float32, kind="ExternalInput")
    wl2bc = nc.dram_tensor("wl2bc", [128, H3], dt.float32, kind="ExternalInput")
    bl2c = nc.dram_tensor("bl2c", [128, 1], dt.float32, kind="ExternalInput")
    esrc = nc.dram_tensor("esrc", [128, CHT], dt.int32, kind="ExternalInput")
    colloc = nc.dram_tensor("colloc", [128, CHT], dt.float32, kind="ExternalInput")
    hsrc0 = nc.dram_tensor("hsrc0", [128, NHC], dt.int32, kind="ExternalInput")
    hsrc1 = nc.dram_tensor("hsrc1", [128, NHC], dt.int32, kind="ExternalInput")
    out_head = nc.dram_tensor("out_head", [128, NHC], dt.float32,
                              kind="ExternalOutput")

    from concourse.masks import make_identity

    with tile.TileContext(nc) as tc:
        with tc.tile_pool(name="const", bufs=1) as cpool, \
             tc.tile_pool(name="dram", bufs=1, space="DRAM") as dpool, \
             tc.tile_pool(name="gat", bufs=8) as gat_pool, \
             tc.tile_pool(name="ind", bufs=8) as ind_pool, \
             tc.tile_pool(name="work", bufs=4) as work, \
             tc.tile_pool(name="psA", bufs=2, space="PSUM") as psA, \
             tc.tile_pool(name="psT", bufs=2, space="PSUM") as psT:

            # ---- constants / index preload ----
            ident = cpool.tile([128, 128], dt.float32)
            make_identity(nc, ident[:])
            iota_i = cpool.tile([128, 128], dt.int32)
            nc.gpsimd.iota(iota_i[:], pattern=[[1, 128]], base=0,
                           channel_multiplier=0)
            iota_f = cpool.tile([128, 128], dt.float32)
            nc.vector.tensor_copy(out=iota_f[:], in_=iota_i[:])
            ident2 = cpool.tile([128, 128], dt.float32)
            nc.vector.tensor_scalar_mul(ident2[:], ident[:], float(SELF_LOOP_W))

            W1s = cpool.tile([128, 2 * H1], dt.float32)
            nc.sync.dma_start(out=W1s[:, :H1], in_=W1[0:128, :])
            nc.sync.dma_start(out=W1s[:, H1:], in_=W1[128:256, :])
            W2s = cpool.tile([128, 2 * H2], dt.float32)
            nc.sync.dma_start(out=W2s[:, :H2], in_=W2[0:128, :])
            nc.sync.dma_start(out=W2s[:, H2:], in_=W2[128:256, :])
            Wl1s = cpool.tile([128, H3], dt.float32)
            nc.sync.dma_start(out=Wl1s[:], in_=Wl1[:])
            b1s = cpool.tile([128, H1], dt.float32)
            nc.sync.dma_start(out=b1s[:], in_=b1t[:])
            b2s = cpool.tile([128, H2], dt.float32)
            nc.sync.dma_start(out=b2s[:], in_=b2t[:])
            bl1s = cpool.tile([128, H3], dt.float32)
            nc.sync.dma_start(out=bl1s[:], in_=bl1t[:])
            wl2s = cpool.tile([128, H3], dt.float32)
            nc.sync.dma_start(out=wl2s[:], in_=wl2bc[:])
            bl2s = cpool.tile([128, 1], dt.float32)
            nc.sync.dma_start(out=bl2s[:], in_=bl2c[:])
            dinv_s = cpool.tile([128, NT], dt.float32)
            nc.sync.dma_start(out=dinv_s[:], in_=dinv_pm[:])
            esrc_s = cpool.tile([128, CHT], dt.int32)
            nc.sync.dma_start(out=esrc_s[:], in_=esrc[:])
            colloc_s = cpool.tile([128, CHT], dt.float32)
            nc.sync.dma_start(out=colloc_s[:], in_=colloc[:])
            h0_s = cpool.tile([128, NHC], dt.int32)
            nc.sync.dma_start(out=h0_s[:], in_=hsrc0[:])
            h1_s = cpool.tile([128, NHC], dt.int32)
            nc.sync.dma_start(out=h1_s[:], in_=hsrc1[:])

            # ---- DRAM internals ----
            g1_loc = dpool.tile([NS, H1], dt.float32)
            gtab1 = dpool.tile([N, H1], dt.float32, addr_space="Shared")
            g2_loc = dpool.tile([NS, H2], dt.float32)
            gtab2 = dpool.tile([N, H2], dt.float32, addr_space="Shared")
            z_loc = dpool.tile([NS, H3], dt.float32)
            ztab = dpool.tile([N, H3], dt.float32, addr_space="Shared")

            rg = [list(range(N_CORES))]

            # ================= Phase A: g1 shard =================
            for t in range(NT):
                rows = min(128, NS - t * 128)
                xa = work.tile([128, 128], dt.float32, tag="xa")
                xb = work.tile([128, 128], dt.float32, tag="xb")
                nc.sync.dma_start(out=xa[:], in_=xT[0:128, t * 128:(t + 1) * 128])
                nc.sync.dma_start(out=xb[:], in_=xT[128:256, t * 128:(t + 1) * 128])
                ps = psA.tile([128, H1], dt.float32, tag="psagg")
                nc.tensor.matmul(out=ps[:], lhsT=xa[:], rhs=W1s[:, :H1],
                                 start=True, stop=False)
                nc.tensor.matmul(out=ps[:], lhsT=xb[:], rhs=W1s[:, H1:],
                                 start=False, stop=True)
                g1v = work.tile([128, H1], dt.float32, tag="g1v")
                nc.vector.tensor_mul(
                    out=g1v[:], in0=ps[:],
                    in1=dinv_s[:, t:t + 1].to_broadcast([128, H1]))
                nc.sync.dma_start(out=g1_loc[t * 128: t * 128 + rows, :],
                                  in_=g1v[:rows, :])
            nc.gpsimd.collective_compute(
                "AllGather", mybir.AluOpType.bypass, replica_groups=rg,
                ins=[g1_loc.opt()], outs=[gtab1.opt()])

            # ============ Layer helpers ============
            def agg_layer(gtab, F, gloc):
                """Yields per-tile psum [128, F]: self-loop (x2, direct DMA
                load from the core-local shard) + Cts[t] gathered chunks."""
                for t in range(NT):
                    rows = min(128, NS - t * 128)
                    ps = psA.tile([128, F], dt.float32, tag="psagg")
                    # self chunk: contiguous rows of own shard, indicator 2*I
                    gs = gat_pool.tile([128, F], dt.float32, tag="gath")
                    nc.sync.dma_start(
                        out=gs[:rows, :],
                        in_=gloc[t * 128: t * 128 + rows, :])
                    nc.tensor.matmul(out=ps[:], lhsT=ident2[:], rhs=gs[:],
                                     start=True, stop=False)
                    for c in range(Cts[t]):
                        j = int(bases[t]) + c
                        g = gat_pool.tile([128, F], dt.float32, tag="gath")
                        nc.gpsimd.indirect_dma_start(
                            out=g[:], out_offset=None, in_=gtab[:],
                            in_offset=bass.IndirectOffsetOnAxis(
                                ap=esrc_s[:, j:j + 1], axis=0))
                        ind = ind_pool.tile([128, 128], dt.float32, tag="ind")
                        nc.vector.tensor_tensor(
                            out=ind[:],
                            in0=colloc_s[:, j:j + 1].to_broadcast([128, 128]),
                            in1=iota_f[:], op=mybir.AluOpType.is_equal)
                        nc.tensor.matmul(out=ps[:], lhsT=ind[:], rhs=g[:],
                                         start=False, stop=(c == Cts[t] - 1))
                    yield t, ps

            # ============ Layer 1 + fused g2 ============
            for t, ps in agg_layer(gtab1, H1, g1_loc):
                rows = min(128, NS - t * 128)
                dv = dinv_s[:, t:t + 1]
                h1v = work.tile([128, H1], dt.float32, tag="h1v")
                nc.vector.tensor_mul(out=h1v[:], in0=ps[:],
                                     in1=dv.to_broadcast([128, H1]))
                nc.vector.tensor_add(out=h1v[:], in0=h1v[:], in1=b1s[:])
                nc.scalar.activation(out=h1v[:], in_=h1v[:],
                                     func=mybir.ActivationFunctionType.Relu)
                nc.vector.tensor_mul(out=h1v[:], in0=h1v[:],
                                     in1=dv.to_broadcast([128, H1]))
                # transpose h1d -> [feat, rows], then g2 = h1d @ W2
                g2ps = psA.tile([128, H2], dt.float32, tag="pssm")
                tpss = []
                for fb in range(2):
                    tp = psT.tile([128, 128], dt.float32, tag="tp")
                    nc.tensor.transpose(out=tp[:],
                                        in_=h1v[:, fb * 128:(fb + 1) * 128],
                                        identity=ident[:])
                    tps = work.tile([128, 128], dt.float32, tag=f"tps{fb}")
                    nc.vector.tensor_copy(out=tps[:], in_=tp[:])
                    tpss.append(tps)
                for fb in range(2):
                    nc.tensor.matmul(out=g2ps[:], lhsT=tpss[fb][:],
                                     rhs=W2s[:, fb * H2:(fb + 1) * H2],
                                     start=(fb == 0), stop=(fb == 1))
                g2v = work.tile([128, H2], dt.float32, tag="g2v")
                nc.vector.tensor_copy(out=g2v[:], in_=g2ps[:])
                nc.sync.dma_start(out=g2_loc[t * 128: t * 128 + rows, :],
                                  in_=g2v[:rows, :])
            nc.gpsimd.collective_compute(
                "AllGather", mybir.AluOpType.bypass, replica_groups=rg,
                ins=[g2_loc.opt()], outs=[gtab2.opt()])

            # ============ Layer 2 + fused z ============
            for t, ps in agg_layer(gtab2, H2, g2_loc):
                rows = min(128, NS - t * 128)
                dv = dinv_s[:, t:t + 1]
                h2v = work.tile([128, H2], dt.float32, tag="h2v")
                nc.vector.tensor_mul(out=h2v[:], in0=ps[:],
                                     in1=dv.to_broadcast([128, H2]))
                nc.vector.tensor_add(out=h2v[:], in0=h2v[:], in1=b2s[:])
                tp = psT.tile([128, 128], dt.float32, tag="tp")
                nc.tensor.transpose(out=tp[:], in_=h2v[:], identity=ident[:])
                tps = work.tile([128, 128], dt.float32, tag="tps")
                nc.vector.tensor_copy(out=tps[:], in_=tp[:])
                zps = psA.tile([128, H3], dt.float32, tag="pssm")
                nc.tensor.matmul(out=zps[:], lhsT=tps[:], rhs=Wl1s[:],
                                 start=True, stop=True)
                zv = work.tile([128, H3], dt.float32, tag="zv")
                nc.vector.tensor_copy(out=zv[:], in_=zps[:])
                nc.sync.dma_start(out=z_loc[t * 128: t * 128 + rows, :],
                                  in_=zv[:rows, :])
            nc.gpsimd.collective_compute(
                "AllGather", mybir.AluOpType.bypass, replica_groups=rg,
                ins=[z_loc.opt()], outs=[ztab.opt()])

            # ============ Edge head ============
            out_sb = cpool.tile([128, NHC], dt.float32)
            for c in range(NHC):
                r0 = gat_pool.tile([128, H3], dt.float32, tag="hg0")
                nc.gpsimd.indirect_dma_start(
                    out=r0[:], out_offset=None, in_=ztab[:],
                    in_offset=bass.IndirectOffsetOnAxis(
                        ap=h0_s[:, c:c + 1], axis=0))
                r1 = gat_pool.tile([128, H3], dt.float32, tag="hg1")
                nc.gpsimd.indirect_dma_start(
                    out=r1[:], out_offset=None, in_=ztab[:],
                    in_offset=bass.IndirectOffsetOnAxis(
                        ap=h1_s[:, c:c + 1], axis=0))
                e1 = work.tile([128, H3], dt.float32, tag="e1")
                nc.vector.tensor_add(out=e1[:], in0=r0[:], in1=r1[:])
                nc.vector.tensor_add(out=e1[:], in0=e1[:], in1=bl1s[:])
                nc.scalar.activation(out=e1[:], in_=e1[:],
                                     func=mybir.ActivationFunctionType.Relu)
                nc.vector.tensor_mul(out=e1[:], in0=e1[:], in1=wl2s[:])
                sc = work.tile([128, 1], dt.float32, tag="sc")
                nc.vector.reduce_sum(out=sc[:], in_=e1[:],
                                     axis=mybir.AxisListType.X)
                nc.scalar.activation(out=out_sb[:, c:c + 1], in_=sc[:],
                                     func=mybir.ActivationFunctionType.Sigmoid,
                                     bias=bl2s[:])
            nc.sync.dma_start(out=out_head[:], in_=out_sb[:])

    nc.compile()
    return nc


def _prep_inputs(x, train_edge_index, pos_edge_index, neg_edge_index,
                 W1, b1, W2, b2, Wl1, bl1, Wl2, bl2):
    """Host-side sharding / index layout. Returns (in_maps, C)."""
    x = np.asarray(x, np.float32)
    ei = np.asarray(train_edge_index)
    row, col = ei[0].astype(np.int64), ei[1].astype(np.int64)
    deg = np.bincount(col, minlength=N).astype(np.float32) + SELF_LOOP_W
    dinv = (1.0 / np.sqrt(deg)).astype(np.float32)

    W1 = np.asarray(W1, np.float32)
    W2 = np.asarray(W2, np.float32)
    Wl1 = np.asarray(Wl1, np.float32)
    b1 = np.asarray(b1, np.float32)
    b2 = np.asarray(b2, np.float32)
    bl1 = np.asarray(bl1, np.float32)
    Wl2 = np.asarray(Wl2, np.float32).reshape(-1)
    bl2 = np.asarray(bl2, np.float32).reshape(-1)

    # --- per-(core,tile) edge grouping ---
    core_of = col // NS
    tile_of = (col % NS) // 128
    # chunk requirement per (core, tile)
    counts = np.zeros((N_CORES, NT), np.int64)
    np.add.at(counts, (core_of, tile_of), 1)
    Cts = [int(np.ceil(counts[:, t].max() / 128.0)) for t in range(NT)]
    bases = np.cumsum([0] + Cts)[:-1]
    CHT = int(sum(Cts))

    order = np.lexsort((tile_of, core_of))
    row_s, col_s = row[order], col[order]
    core_s, tile_s = core_of[order], tile_of[order]
    # boundaries per (core,tile)
    grp = core_s * NT + tile_s
    starts = np.searchsorted(grp, np.arange(N_CORES * NT))
    ends = np.searchsorted(grp, np.arange(N_CORES * NT), side='right')

    tei = np.concatenate([np.asarray(pos_edge_index),
                          np.asarray(neg_edge_index)], axis=-1)
    t0_all, t1_all = tei[0].astype(np.int64), tei[1].astype(np.int64)

    in_maps = []
    for k in range(N_CORES):
        lo = k * NS
        esrc = np.zeros((128, CHT), np.int32)
        colloc = np.full((128, CHT), -1.0, np.float32)
        for t in range(NT):
            Ct = Cts[t]
            base = int(bases[t])
            s, e = starts[k * NT + t], ends[k * NT + t]
            ne = e - s
            assert ne <= Ct * 128, "chunk overflow"
            srcs = row_s[s:e]
            locs = (col_s[s:e] - lo - t * 128).astype(np.float32)
            full = np.zeros(Ct * 128, np.int32)
            fullc = np.full(Ct * 128, -1.0, np.float32)
            full[:ne] = srcs
            fullc[:ne] = locs
            esrc[:, base: base + Ct] = full.reshape(Ct, 128).T
            colloc[:, base: base + Ct] = fullc.reshape(Ct, 128).T

        # head edges
        h0 = np.zeros(NHC * 128, np.int32)
        h1 = np.zeros(NHC * 128, np.int32)
        h0[:HE_CORE] = t0_all[k * HE_CORE:(k + 1) * HE_CORE]
        h1[:HE_CORE] = t1_all[k * HE_CORE:(k + 1) * HE_CORE]
        hsrc0 = h0.reshape(NHC, 128).T.copy()
        hsrc1 = h1.reshape(NHC, 128).T.copy()

        # node shard data
        xs = np.zeros((NSP, F_IN), np.float32)
        xs[:NS] = x[lo:lo + NS]
        xT = np.ascontiguousarray(xs.T)
        dpm = np.zeros((128, NT), np.float32)
        dsh = np.zeros(NSP, np.float32)
        dsh[:NS] = dinv[lo:lo + NS]
        dpm[:, :] = dsh.reshape(NT, 128).T

        in_maps.append({
            "xT": xT, "dinv_pm": dpm,
            "W1": W1, "W2": W2, "Wl1": Wl1,
            "b1t": np.tile(b1[None, :], (128, 1)),
            "b2t": np.tile(b2[None, :], (128, 1)),
            "bl1t": np.tile(bl1[None, :], (128, 1)),
            "wl2bc": np.tile(Wl2[None, :], (128, 1)),
            "bl2c": np.full((128, 1), bl2[0], np.float32),
            "esrc": esrc, "colloc": colloc,
            "hsrc0": hsrc0, "hsrc1": hsrc1,
        })
    return in_maps, tuple(Cts)


def _get_runner(C, in_maps):
    import jax
    from concourse import bass2jax, mybir as mb
    from jax.sharding import Mesh, PartitionSpec
    from jax.experimental.shard_map import shard_map

    key = ("runner", C)
    if key in _CACHE:
        return _CACHE[key]

    nc = _CACHE.get(("nc", C))
    if nc is None:
        nc = _build_and_compile(C)
        _CACHE[("nc", C)] = nc

    bass2jax.install_neuronx_cc_hook()
    partition_name = nc.partition_id_tensor.name if nc.partition_id_tensor else None
    in_names, out_names, out_avals, zero_outs = [], [], [], []
    for a in nc.m.functions[0].allocations:
        if not isinstance(a, mb.MemoryLocationSet):
            continue
        name = a.memorylocations[0].name
        if a.kind == "ExternalInput":
            if name != partition_name:
                in_names.append(name)
        elif a.kind == "ExternalOutput":
            out_names.append(name)
            shape = tuple(a.tensor_shape)
            dtype = mb.dt.np(a.dtype)
            out_avals.append(jax.core.ShapedArray(shape, dtype))
            zero_outs.append(np.zeros(shape, dtype))
    n_params = len(in_names)
    all_in_names = in_names + out_names + ([partition_name] if partition_name else [])

    def _body(*args):
        operands = list(args)
        if partition_name is not None:
            operands.append(bass2jax.partition_id_tensor())
        outs = bass2jax._bass_exec_p.bind(
            *operands, out_avals=tuple(out_avals), in_names=tuple(all_in_names),
            out_names=tuple(out_names), lowering_input_output_aliases=(),
            sim_require_finite=True, sim_require_nnan=True, nc=nc)
        return tuple(outs)

    devices = jax.devices()[:N_CORES]
    mesh = Mesh(np.asarray(devices), ("core",))
    in_specs = (PartitionSpec("core"),) * (n_params + len(out_names))
    out_specs = (PartitionSpec("core"),) * len(out_names)
    sharded = jax.jit(shard_map(_body, mesh=mesh, in_specs=in_specs,
                                out_specs=out_specs, check_rep=False),
                      keep_unused=True)

    def run(maps):
        concat_in = [np.concatenate([np.asarray(maps[c][nm])
                                     for c in range(N_CORES)], axis=0)
                     for nm in in_names]
        concat_zero = [np.concatenate([z] * N_CORES, axis=0) for z in zero_outs]
        outs = sharded(*concat_in, *concat_zero)
        jax.block_until_ready(outs)
        return {nm: np.asarray(outs[i]) for i, nm in enumerate(out_names)}

    _CACHE[key] = run
    return run


def kernel(**inputs) -> np.ndarray:
    in_maps, C = _prep_inputs(**inputs)
    run = _get_runner(C, in_maps)
    outs = run(in_maps)
    oh = outs["out_head"].reshape(N_CORES, 128, NHC)
    res = np.empty(HEAD_E, np.float32)
    for k in range(N_CORES):
        flat = oh[k].T.reshape(-1)  # [NHC*128] in edge order
        res[k * HE_CORE:(k + 1) * HE_CORE] = flat[:HE_CORE]
    return res


# revision 9
# speedup vs baseline: 181.4048x; 1.1870x over previous
"""GCN link-prediction kernel for 8 Trainium2 NeuronCores.

Strategy (target-sharded edges, replicated-by-AllGather node tables):
  - Nodes split into 8 contiguous shards. Each core computes its shard of
    g1 = dinv * (x @ W1) on PE, AllGather -> full table gtab1 in every
    core's HBM.
  - Train edges assigned to the core owning the TARGET node, grouped per
    128-target tile, padded to a fixed chunk count. Per 128-edge chunk:
    indirect-DMA gather of source rows, a DVE is_equal indicator matrix
    [edges x targets], and one PE matmul accumulating into PSUM.
    Self-loops (weight 2) are a per-tile extra chunk whose indicator is
    doubled.
  - Layer epilogue fuses dinv scaling, bias, relu, and the next layer's
    dense matmul (transposed via PE) so h1/h2 never round-trip to DRAM.
  - Edge head: z = h2 @ Wl1 table (64 f32), two gathers per 128-edge
    chunk, elementwise ops + free-dim reduction + sigmoid.

All float math runs on device in fp32; host only reorders/pads integer
edge indices and computes dinv (index-derived degree scaling).
"""
import sys
import os
import numpy as np

sys.path.insert(0, '/opt/trn_rl_repo')

N_CORES = 8
N = 50000
F_IN, H1, H2, H3 = 256, 256, 128, 64
SELF_LOOP_W = 2.0

NS = N // N_CORES            # 6250 nodes per shard
NT = (NS + 127) // 128       # 49 target tiles per core
NSP = NT * 128               # 6272 padded shard rows
HEAD_E = 400000
HE_CORE = HEAD_E // N_CORES  # 50000 head edges per core
NHC = (HE_CORE + 127) // 128  # 391 head chunks

_CACHE = {}


def _build_and_compile(Cts, variant='full'):
    """Build the SPMD Bass program. Cts[t] = data-chunks for target tile t."""
    import concourse.bass as bass
    import concourse.mybir as mybir
    import concourse.tile as tile
    from concourse import bacc

    dt = mybir.dt
    Cts = list(Cts)
    CHT = sum(Cts)        # data chunks per core (self-loop via direct DMA)
    bases = np.cumsum([0] + Cts)[:-1]

    nc = bacc.Bacc("TRN2", target_bir_lowering=False, debug=False,
                   num_devices=N_CORES)

    # ---- I/O ----
    xT = nc.dram_tensor("xT", [F_IN, NSP], dt.float32, kind="ExternalInput")
    dinv_pm = nc.dram_tensor("dinv_pm", [128, NT], dt.float32, kind="ExternalInput")
    W1 = nc.dram_tensor("W1", [F_IN, H1], dt.float32, kind="ExternalInput")
    W2 = nc.dram_tensor("W2", [H1, H2], dt.float32, kind="ExternalInput")
    Wl1 = nc.dram_tensor("Wl1", [H2, H3], dt.float32, kind="ExternalInput")
    b1t = nc.dram_tensor("b1t", [128, H1], dt.float32, kind="ExternalInput")
    b2t = nc.dram_tensor("b2t", [128, H2], dt.float32, kind="ExternalInput")
    bl1t = nc.dram_tensor("bl1t", [128, H3], dt.float32, kind="ExternalInput")
    wl2bc = nc.dram_tensor("wl2bc", [128, H3], dt.float32, kind="ExternalInput")
    bl2c = nc.dram_tensor("bl2c", [128, 1], dt.float32, kind="ExternalInput")
    esrc = nc.dram_tensor("esrc", [128, CHT], dt.int32, kind="ExternalInput")
    colloc = nc.dram_tensor("colloc", [128, CHT], dt.float32, kind="ExternalInput")
    hsrc0 = nc.dram_tensor("hsrc0", [128, NHC], dt.int32, kind="ExternalInput")
    hsrc1 = nc.dram_tensor("hsrc1", [128, NHC], dt.int32, kind="ExternalInput")
    out_head = nc.dram_tensor("out_head", [128, NHC], dt.float32,
                              kind="ExternalOutput")

    from concourse.masks import make_identity

    with tile.TileContext(nc) as tc:
        with tc.tile_pool(name="const", bufs=1) as cpool, \
             tc.tile_pool(name="dram", bufs=1, space="DRAM") as dpool, \
             tc.tile_pool(name="gat", bufs=12) as gat_pool, \
             tc.tile_pool(name="ind", bufs=12) as ind_pool, \
             tc.tile_pool(name="work", bufs=6) as work, \
             tc.tile_pool(name="psA", bufs=3, space="PSUM") as psA, \
             tc.tile_pool(name="psT", bufs=2, space="PSUM") as psT:

            # ---- constants / index preload ----
            ident = cpool.tile([128, 128], dt.float32)
            make_identity(nc, ident[:])
            iota_i = cpool.tile([128, 128], dt.int32)
            nc.gpsimd.iota(iota_i[:], pattern=[[1, 128]], base=0,
                           channel_multiplier=0)
            iota_f = cpool.tile([128, 128], dt.float32)
            nc.vector.tensor_copy(out=iota_f[:], in_=iota_i[:])
            ident2 = cpool.tile([128, 128], dt.float32)
            nc.vector.tensor_scalar_mul(ident2[:], ident[:], float(SELF_LOOP_W))

            W1s = cpool.tile([128, 2 * H1], dt.float32)
            nc.sync.dma_start(out=W1s[:, :H1], in_=W1[0:128, :])
            nc.sync.dma_start(out=W1s[:, H1:], in_=W1[128:256, :])
            W2s = cpool.tile([128, 2 * H2], dt.float32)
            nc.sync.dma_start(out=W2s[:, :H2], in_=W2[0:128, :])
            nc.sync.dma_start(out=W2s[:, H2:], in_=W2[128:256, :])
            Wl1s = cpool.tile([128, H3], dt.float32)
            nc.sync.dma_start(out=Wl1s[:], in_=Wl1[:])
            b1s = cpool.tile([128, H1], dt.float32)
            nc.sync.dma_start(out=b1s[:], in_=b1t[:])
            b2s = cpool.tile([128, H2], dt.float32)
            nc.sync.dma_start(out=b2s[:], in_=b2t[:])
            bl1s = cpool.tile([128, H3], dt.float32)
            nc.sync.dma_start(out=bl1s[:], in_=bl1t[:])
            wl2s = cpool.tile([128, H3], dt.float32)
            nc.sync.dma_start(out=wl2s[:], in_=wl2bc[:])
            bl2s = cpool.tile([128, 1], dt.float32)
            nc.sync.dma_start(out=bl2s[:], in_=bl2c[:])
            dinv_s = cpool.tile([128, NT], dt.float32)
            nc.sync.dma_start(out=dinv_s[:], in_=dinv_pm[:])
            esrc_s = cpool.tile([128, CHT], dt.int32)
            nc.sync.dma_start(out=esrc_s[:], in_=esrc[:])
            colloc_s = cpool.tile([128, CHT], dt.float32)
            nc.sync.dma_start(out=colloc_s[:], in_=colloc[:])
            h0_s = cpool.tile([128, NHC], dt.int32)
            nc.sync.dma_start(out=h0_s[:], in_=hsrc0[:])
            h1_s = cpool.tile([128, NHC], dt.int32)
            nc.sync.dma_start(out=h1_s[:], in_=hsrc1[:])

            # ---- DRAM internals ----
            g1_loc = dpool.tile([NS, H1], dt.float32)
            gtab1 = dpool.tile([N, H1], dt.float32, addr_space="Shared")
            g2_loc = dpool.tile([NS, H2], dt.float32)
            gtab2 = dpool.tile([N, H2], dt.float32, addr_space="Shared")
            z_loc = dpool.tile([NS, H3], dt.float32)
            ztab = dpool.tile([N, H3], dt.float32, addr_space="Shared")

            rg = [list(range(N_CORES))]

            # ================= Phase A: g1 shard =================
            for t in range(NT):
                rows = min(128, NS - t * 128)
                xa = work.tile([128, 128], dt.float32, tag="xa")
                xb = work.tile([128, 128], dt.float32, tag="xb")
                nc.sync.dma_start(out=xa[:], in_=xT[0:128, t * 128:(t + 1) * 128])
                nc.sync.dma_start(out=xb[:], in_=xT[128:256, t * 128:(t + 1) * 128])
                ps = psA.tile([128, H1], dt.float32, tag="psagg")
                nc.tensor.matmul(out=ps[:], lhsT=xa[:], rhs=W1s[:, :H1],
                                 start=True, stop=False)
                nc.tensor.matmul(out=ps[:], lhsT=xb[:], rhs=W1s[:, H1:],
                                 start=False, stop=True)
                g1v = work.tile([128, H1], dt.float32, tag="g1v")
                nc.vector.tensor_mul(
                    out=g1v[:], in0=ps[:],
                    in1=dinv_s[:, t:t + 1].to_broadcast([128, H1]))
                nc.sync.dma_start(out=g1_loc[t * 128: t * 128 + rows, :],
                                  in_=g1v[:rows, :])
            nc.gpsimd.collective_compute(
                "AllGather", mybir.AluOpType.bypass, replica_groups=rg,
                ins=[g1_loc.opt()], outs=[gtab1.opt()])

            # ============ Layer helpers ============
            def agg_layer(gtab, F, gloc):
                """Yields per-tile psum [128, F]: self-loop (x2, direct DMA
                load from the core-local shard) + Cts[t] gathered chunks."""
                for t in range(NT):
                    rows = min(128, NS - t * 128)
                    ps = psA.tile([128, F], dt.float32, tag="psagg")
                    # self chunk: contiguous rows of own shard, indicator 2*I
                    gs = gat_pool.tile([128, F], dt.float32, tag="gath")
                    nc.sync.dma_start(
                        out=gs[:rows, :],
                        in_=gloc[t * 128: t * 128 + rows, :])
                    nc.tensor.matmul(out=ps[:], lhsT=ident2[:], rhs=gs[:],
                                     start=True, stop=False)
                    for c in range(0 if variant == 'noagg' else Cts[t]):
                        j = int(bases[t]) + c
                        g = gat_pool.tile([128, F], dt.float32, tag="gath")
                        nc.gpsimd.indirect_dma_start(
                            out=g[:], out_offset=None, in_=gtab[:],
                            in_offset=bass.IndirectOffsetOnAxis(
                                ap=esrc_s[:, j:j + 1], axis=0))
                        ind = ind_pool.tile([128, 128], dt.float32, tag="ind")
                        nc.vector.tensor_tensor(
                            out=ind[:],
                            in0=colloc_s[:, j:j + 1].to_broadcast([128, 128]),
                            in1=iota_f[:], op=mybir.AluOpType.is_equal)
                        nc.tensor.matmul(out=ps[:], lhsT=ind[:], rhs=g[:],
                                         start=False, stop=(c == Cts[t] - 1))
                    yield t, ps

            # ============ Layer 1 + fused g2 ============
            for t, ps in agg_layer(gtab1, H1, g1_loc):
                rows = min(128, NS - t * 128)
                dv = dinv_s[:, t:t + 1]
                h1v = work.tile([128, H1], dt.float32, tag="h1v")
                nc.vector.tensor_mul(out=h1v[:], in0=ps[:],
                                     in1=dv.to_broadcast([128, H1]))
                nc.vector.tensor_add(out=h1v[:], in0=h1v[:], in1=b1s[:])
                nc.scalar.activation(out=h1v[:], in_=h1v[:],
                                     func=mybir.ActivationFunctionType.Relu)
                nc.vector.tensor_mul(out=h1v[:], in0=h1v[:],
                                     in1=dv.to_broadcast([128, H1]))
                # transpose h1d -> [feat, rows], then g2 = h1d @ W2
                g2ps = psA.tile([128, H2], dt.float32, tag="pssm")
                tpss = []
                for fb in range(2):
                    tp = psT.tile([128, 128], dt.float32, tag="tp")
                    nc.tensor.transpose(out=tp[:],
                                        in_=h1v[:, fb * 128:(fb + 1) * 128],
                                        identity=ident[:])
                    tps = work.tile([128, 128], dt.float32, tag=f"tps{fb}")
                    nc.vector.tensor_copy(out=tps[:], in_=tp[:])
                    tpss.append(tps)
                for fb in range(2):
                    nc.tensor.matmul(out=g2ps[:], lhsT=tpss[fb][:],
                                     rhs=W2s[:, fb * H2:(fb + 1) * H2],
                                     start=(fb == 0), stop=(fb == 1))
                g2v = work.tile([128, H2], dt.float32, tag="g2v")
                nc.vector.tensor_copy(out=g2v[:], in_=g2ps[:])
                nc.sync.dma_start(out=g2_loc[t * 128: t * 128 + rows, :],
                                  in_=g2v[:rows, :])
            nc.gpsimd.collective_compute(
                "AllGather", mybir.AluOpType.bypass, replica_groups=rg,
                ins=[g2_loc.opt()], outs=[gtab2.opt()])

            # ============ Layer 2 + fused z ============
            for t, ps in agg_layer(gtab2, H2, g2_loc):
                rows = min(128, NS - t * 128)
                dv = dinv_s[:, t:t + 1]
                h2v = work.tile([128, H2], dt.float32, tag="h2v")
                nc.vector.tensor_mul(out=h2v[:], in0=ps[:],
                                     in1=dv.to_broadcast([128, H2]))
                nc.vector.tensor_add(out=h2v[:], in0=h2v[:], in1=b2s[:])
                tp = psT.tile([128, 128], dt.float32, tag="tp")
                nc.tensor.transpose(out=tp[:], in_=h2v[:], identity=ident[:])
                tps = work.tile([128, 128], dt.float32, tag="tps")
                nc.vector.tensor_copy(out=tps[:], in_=tp[:])
                zps = psA.tile([128, H3], dt.float32, tag="pssm")
                nc.tensor.matmul(out=zps[:], lhsT=tps[:], rhs=Wl1s[:],
                                 start=True, stop=True)
                zv = work.tile([128, H3], dt.float32, tag="zv")
                nc.vector.tensor_copy(out=zv[:], in_=zps[:])
                nc.sync.dma_start(out=z_loc[t * 128: t * 128 + rows, :],
                                  in_=zv[:rows, :])
            nc.gpsimd.collective_compute(
                "AllGather", mybir.AluOpType.bypass, replica_groups=rg,
                ins=[z_loc.opt()], outs=[ztab.opt()])

            # ============ Edge head ============
            out_sb = cpool.tile([128, NHC], dt.float32)
            if variant == 'nohead':
                nc.gpsimd.memset(out_sb[:], 0)
            for c in range(0 if variant == 'nohead' else NHC):
                r0 = gat_pool.tile([128, H3], dt.float32, tag="hg0")
                nc.gpsimd.indirect_dma_start(
                    out=r0[:], out_offset=None, in_=ztab[:],
                    in_offset=bass.IndirectOffsetOnAxis(
                        ap=h0_s[:, c:c + 1], axis=0))
                r1 = gat_pool.tile([128, H3], dt.float32, tag="hg1")
                nc.gpsimd.indirect_dma_start(
                    out=r1[:], out_offset=None, in_=ztab[:],
                    in_offset=bass.IndirectOffsetOnAxis(
                        ap=h1_s[:, c:c + 1], axis=0))
                e1 = work.tile([128, H3], dt.float32, tag="e1")
                nc.vector.tensor_add(out=e1[:], in0=r0[:], in1=r1[:])
                nc.vector.tensor_add(out=e1[:], in0=e1[:], in1=bl1s[:])
                nc.scalar.activation(out=e1[:], in_=e1[:],
                                     func=mybir.ActivationFunctionType.Relu)
                nc.vector.tensor_mul(out=e1[:], in0=e1[:], in1=wl2s[:])
                sc = work.tile([128, 1], dt.float32, tag="sc")
                nc.vector.reduce_sum(out=sc[:], in_=e1[:],
                                     axis=mybir.AxisListType.X)
                nc.scalar.activation(out=out_sb[:, c:c + 1], in_=sc[:],
                                     func=mybir.ActivationFunctionType.Sigmoid,
                                     bias=bl2s[:])
            nc.sync.dma_start(out=out_head[:], in_=out_sb[:])

    nc.compile()
    return nc


def _prep_inputs(x, train_edge_index, pos_edge_index, neg_edge_index,
                 W1, b1, W2, b2, Wl1, bl1, Wl2, bl2):
    """Host-side sharding / index layout. Returns (in_maps, C)."""
    x = np.asarray(x, np.float32)
    ei = np.asarray(train_edge_index)
    row, col = ei[0].astype(np.int64), ei[1].astype(np.int64)
    deg = np.bincount(col, minlength=N).astype(np.float32) + SELF_LOOP_W
    dinv = (1.0 / np.sqrt(deg)).astype(np.float32)

    W1 = np.asarray(W1, np.float32)
    W2 = np.asarray(W2, np.float32)
    Wl1 = np.asarray(Wl1, np.float32)
    b1 = np.asarray(b1, np.float32)
    b2 = np.asarray(b2, np.float32)
    bl1 = np.asarray(bl1, np.float32)
    Wl2 = np.asarray(Wl2, np.float32).reshape(-1)
    bl2 = np.asarray(bl2, np.float32).reshape(-1)

    # --- per-(core,tile) edge grouping ---
    core_of = col // NS
    tile_of = (col % NS) // 128
    # chunk requirement per (core, tile)
    counts = np.zeros((N_CORES, NT), np.int64)
    np.add.at(counts, (core_of, tile_of), 1)
    Cts = [int(np.ceil(counts[:, t].max() / 128.0)) for t in range(NT)]
    bases = np.cumsum([0] + Cts)[:-1]
    CHT = int(sum(Cts))

    order = np.lexsort((tile_of, core_of))
    row_s, col_s = row[order], col[order]
    core_s, tile_s = core_of[order], tile_of[order]
    # boundaries per (core,tile)
    grp = core_s * NT + tile_s
    starts = np.searchsorted(grp, np.arange(N_CORES * NT))
    ends = np.searchsorted(grp, np.arange(N_CORES * NT), side='right')

    tei = np.concatenate([np.asarray(pos_edge_index),
                          np.asarray(neg_edge_index)], axis=-1)
    t0_all, t1_all = tei[0].astype(np.int64), tei[1].astype(np.int64)

    in_maps = []
    for k in range(N_CORES):
        lo = k * NS
        esrc = np.zeros((128, CHT), np.int32)
        colloc = np.full((128, CHT), -1.0, np.float32)
        for t in range(NT):
            Ct = Cts[t]
            base = int(bases[t])
            s, e = starts[k * NT + t], ends[k * NT + t]
            ne = e - s
            assert ne <= Ct * 128, "chunk overflow"
            srcs = row_s[s:e]
            locs = (col_s[s:e] - lo - t * 128).astype(np.float32)
            full = np.zeros(Ct * 128, np.int32)
            fullc = np.full(Ct * 128, -1.0, np.float32)
            full[:ne] = srcs
            fullc[:ne] = locs
            esrc[:, base: base + Ct] = full.reshape(Ct, 128).T
            colloc[:, base: base + Ct] = fullc.reshape(Ct, 128).T

        # head edges
        h0 = np.zeros(NHC * 128, np.int32)
        h1 = np.zeros(NHC * 128, np.int32)
        h0[:HE_CORE] = t0_all[k * HE_CORE:(k + 1) * HE_CORE]
        h1[:HE_CORE] = t1_all[k * HE_CORE:(k + 1) * HE_CORE]
        hsrc0 = h0.reshape(NHC, 128).T.copy()
        hsrc1 = h1.reshape(NHC, 128).T.copy()

        # node shard data
        xs = np.zeros((NSP, F_IN), np.float32)
        xs[:NS] = x[lo:lo + NS]
        xT = np.ascontiguousarray(xs.T)
        dpm = np.zeros((128, NT), np.float32)
        dsh = np.zeros(NSP, np.float32)
        dsh[:NS] = dinv[lo:lo + NS]
        dpm[:, :] = dsh.reshape(NT, 128).T

        in_maps.append({
            "xT": xT, "dinv_pm": dpm,
            "W1": W1, "W2": W2, "Wl1": Wl1,
            "b1t": np.tile(b1[None, :], (128, 1)),
            "b2t": np.tile(b2[None, :], (128, 1)),
            "bl1t": np.tile(bl1[None, :], (128, 1)),
            "wl2bc": np.tile(Wl2[None, :], (128, 1)),
            "bl2c": np.full((128, 1), bl2[0], np.float32),
            "esrc": esrc, "colloc": colloc,
            "hsrc0": hsrc0, "hsrc1": hsrc1,
        })
    return in_maps, tuple(Cts)


def _get_runner(C, in_maps):
    import jax
    from concourse import bass2jax, mybir as mb
    from jax.sharding import Mesh, PartitionSpec
    from jax.experimental.shard_map import shard_map

    key = ("runner", C)
    if key in _CACHE:
        return _CACHE[key]

    nc = _CACHE.get(("nc", C))
    if nc is None:
        nc = _build_and_compile(C)
        _CACHE[("nc", C)] = nc

    bass2jax.install_neuronx_cc_hook()
    partition_name = nc.partition_id_tensor.name if nc.partition_id_tensor else None
    in_names, out_names, out_avals, zero_outs = [], [], [], []
    for a in nc.m.functions[0].allocations:
        if not isinstance(a, mb.MemoryLocationSet):
            continue
        name = a.memorylocations[0].name
        if a.kind == "ExternalInput":
            if name != partition_name:
                in_names.append(name)
        elif a.kind == "ExternalOutput":
            out_names.append(name)
            shape = tuple(a.tensor_shape)
            dtype = mb.dt.np(a.dtype)
            out_avals.append(jax.core.ShapedArray(shape, dtype))
            zero_outs.append(np.zeros(shape, dtype))
    n_params = len(in_names)
    all_in_names = in_names + out_names + ([partition_name] if partition_name else [])

    def _body(*args):
        operands = list(args)
        if partition_name is not None:
            operands.append(bass2jax.partition_id_tensor())
        outs = bass2jax._bass_exec_p.bind(
            *operands, out_avals=tuple(out_avals), in_names=tuple(all_in_names),
            out_names=tuple(out_names), lowering_input_output_aliases=(),
            sim_require_finite=True, sim_require_nnan=True, nc=nc)
        return tuple(outs)

    devices = jax.devices()[:N_CORES]
    mesh = Mesh(np.asarray(devices), ("core",))
    in_specs = (PartitionSpec("core"),) * (n_params + len(out_names))
    out_specs = (PartitionSpec("core"),) * len(out_names)
    sharded = jax.jit(shard_map(_body, mesh=mesh, in_specs=in_specs,
                                out_specs=out_specs, check_rep=False),
                      keep_unused=True)

    def run(maps):
        concat_in = [np.concatenate([np.asarray(maps[c][nm])
                                     for c in range(N_CORES)], axis=0)
                     for nm in in_names]
        concat_zero = [np.concatenate([z] * N_CORES, axis=0) for z in zero_outs]
        outs = sharded(*concat_in, *concat_zero)
        jax.block_until_ready(outs)
        return {nm: np.asarray(outs[i]) for i, nm in enumerate(out_names)}

    _CACHE[key] = run
    return run


def kernel(**inputs) -> np.ndarray:
    in_maps, C = _prep_inputs(**inputs)
    run = _get_runner(C, in_maps)
    outs = run(in_maps)
    oh = outs["out_head"].reshape(N_CORES, 128, NHC)
    res = np.empty(HEAD_E, np.float32)
    for k in range(N_CORES):
        flat = oh[k].T.reshape(-1)  # [NHC*128] in edge order
        res[k * HE_CORE:(k + 1) * HE_CORE] = flat[:HE_CORE]
    return res
